# revision 12
# baseline (speedup 1.0000x reference)
"""Trainium2 Bass kernel for CFGNodeEncoderExpressionUpdateLayer.

Reference computation (per masked node row):
    idx  = nonzero(mask)                     # M rows of N
    prev = previous_cfg_nodes_encodings[idx]            # [M, 256]
    upd  = cfg_combined_expressions_encodings           # [M, 512]
    g    = sigmoid(concat(prev, upd) @ W_forget + b_forget)
    out_rows = g * prev + (1 - g) * (upd @ W_proj + b_proj)
    out  = previous_cfg_nodes_encodings; out[idx] = out_rows

Strategy:
  - Host: gather the M masked prev rows, shard rows across 8 cores
    (M/8 = 25000 rows each), transpose shards to [features, rows] so the
    device does only contiguous DMA and zero on-device transposes, pad
    rows to a multiple of 512.
  - Device (per core): load ~1MB column segments (2048 rows); for each
    512-row sub-block accumulate the two GEMMs over 128-deep K chunks in
    PSUM (prev in fp32r = fp32 bytes at 1 cycle/row; expressions and
    their weights in fp16 to halve their HBM traffic — the PE rounds
    fp32r to ~tf32 precision anyway), sigmoid+bias on ScalarE, and the
    gated blend out = pb - g*(pb - prev), pb = proj + b_proj, on VectorE
    via fused scalar_tensor_tensor ops.  prev and the output stay fp32
    end-to-end.  Tile double-buffers everything; DMA-bound at roughly
    the 358 GB/s per-core HBM limit.
  - Host: transpose shards back and scatter into a copy of the full
    prev tensor.  Unmasked rows never touch the device.
"""

import numpy as np

N_NODES = 400000
M_ROWS = 200000
D_NODE = 256
D_EXPR = 512
D_CAT = D_NODE + D_EXPR

N_CORES = 8
MC = M_ROWS // N_CORES          # masked rows per core
BLK = 512                       # rows per block (matmul moving dim)
NBLK = -(-MC // BLK)            # 49
MC_PAD = NBLK * BLK             # 25088

_CACHE = {}
UPD_FP16 = True   # ship expressions + their weights as fp16
PREV_FP16 = True  # ship prev rows as fp16 (blend reads them directly)
OUT_FP16 = True   # return blended rows as fp16 (host upcasts)


def _build_nc(nblk=NBLK, repeat=1, sup=1536, io_bufs=4, mid_bufs=3,
              psum_bufs=2, out_dma="scalar", upd_dma="sync",
              proj_first=True):
    import concourse.bacc as bacc
    import concourse.mybir as mybir
    import concourse.tile as tile

    f32 = mybir.dt.float32
    f32r = mybir.dt.float32r
    f16 = mybir.dt.float16
    u_dt = f16 if UPD_FP16 else f32r
    p_dt = f16 if PREV_FP16 else f32r
    o_dt = f16 if OUT_FP16 else f32
    ALU = mybir.AluOpType

    mc_pad = nblk * BLK
    nc = bacc.Bacc("TRN2", target_bir_lowering=False, debug=False,
                   num_devices=N_CORES)

    xt_prev = nc.dram_tensor("xt_prev", [D_NODE, mc_pad], p_dt,
                             kind="ExternalInput").ap()
    xt_upd = nc.dram_tensor("xt_upd", [D_EXPR, mc_pad], u_dt,
                            kind="ExternalInput").ap()
    w_forget_p = nc.dram_tensor("w_forget_p", [D_NODE, D_NODE], p_dt,
                                kind="ExternalInput").ap()
    w_forget_u = nc.dram_tensor("w_forget_u", [D_EXPR, D_NODE], u_dt,
                                kind="ExternalInput").ap()
    b_forget = nc.dram_tensor("b_forget", [D_NODE], f32,
                              kind="ExternalInput").ap()
    w_proj = nc.dram_tensor("w_proj", [D_EXPR, D_NODE], u_dt,
                            kind="ExternalInput").ap()
    b_proj = nc.dram_tensor("b_proj", [D_NODE], f32,
                            kind="ExternalInput").ap()
    out_t = nc.dram_tensor("out_t", [D_NODE, mc_pad], o_dt,
                           kind="ExternalOutput").ap()

    KP = D_NODE // 128   # 2 prev K-chunks
    KU = D_EXPR // 128   # 4 upd K-chunks
    KG = KP + KU         # 6 gate K-chunks
    NM = D_NODE // 128   # 2 output-feature chunks

    # column segments of the padded row space (big segments => big DMAs)
    mc_pad = nblk * BLK
    segs = []
    pos = 0
    while pos < mc_pad:
        w = min(sup, mc_pad - pos)
        segs.append((pos, w))
        pos += w

    with tile.TileContext(nc) as tc:
        with (
            tc.tile_pool(name="wpool", bufs=1) as wpool,
            tc.tile_pool(name="io", bufs=io_bufs) as io,
            tc.tile_pool(name="mid", bufs=mid_bufs) as mid,
            tc.tile_pool(name="psum", bufs=psum_bufs, space="PSUM") as pp,
        ):
            # --- preload weights & biases (stay resident in SBUF) ---
            wf_t = {}
            for kc in range(KG):
                for m in range(NM):
                    dt_w = p_dt if kc < KP else u_dt
                    src_w = (w_forget_p[kc * 128:(kc + 1) * 128,
                                        m * 128:(m + 1) * 128]
                             if kc < KP else
                             w_forget_u[(kc - KP) * 128:(kc - KP + 1) * 128,
                                        m * 128:(m + 1) * 128])
                    t = wpool.tile([128, 128], dt_w, tag=f"wf_{kc}_{m}")
                    nc.sync.dma_start(t[:], src_w)
                    wf_t[kc, m] = t
            wp_t = {}
            for kc in range(KU):
                for m in range(NM):
                    t = wpool.tile([128, 128], u_dt, tag=f"wp_{kc}_{m}")
                    nc.sync.dma_start(
                        t[:], w_proj[kc * 128:(kc + 1) * 128,
                                     m * 128:(m + 1) * 128])
                    wp_t[kc, m] = t
            bf_t, bp_t = {}, {}
            for m in range(NM):
                t = wpool.tile([128, 1], f32, tag=f"bf_{m}")
                nc.sync.dma_start(t[:], b_forget[m * 128:(m + 1) * 128]
                                  .unsqueeze(1))
                bf_t[m] = t
                t = wpool.tile([128, 1], f32, tag=f"bp_{m}")
                nc.sync.dma_start(t[:], b_proj[m * 128:(m + 1) * 128]
                                  .unsqueeze(1))
                bp_t[m] = t

            # --- main loop over column segments ---
            for it in range(len(segs) * repeat):
                s, w = segs[it % len(segs)]
                pv = []
                for c in range(KP):
                    t = io.tile([128, sup], p_dt, tag=f"pv{c}")
                    nc.sync.dma_start(
                        t[:, :w], xt_prev[c * 128:(c + 1) * 128, s:s + w])
                    pv.append(t)
                up = []
                for c in range(KU):
                    if upd_dma == "scalar":
                        upd_eng = nc.scalar
                    elif upd_dma == "split":
                        upd_eng = nc.scalar if c % 2 else nc.sync
                    else:
                        upd_eng = nc.sync
                    t = io.tile([128, sup], u_dt, tag=f"up{c}")
                    upd_eng.dma_start(
                        t[:, :w], xt_upd[c * 128:(c + 1) * 128, s:s + w])
                    up.append(t)
                ot = [io.tile([128, sup], o_dt, tag=f"o{m}", name=f"ot{m}")
                      for m in range(NM)]
                rhs_gate = pv + up
                for j in range(w // BLK):
                    js = slice(j * BLK, (j + 1) * BLK)
                    for m in range(NM):
                        pg = pp.tile([128, BLK], f32, tag=f"pg{m}")
                        pj = pp.tile([128, BLK], f32, tag=f"pp{m}")

                        def emit_gate():
                            for kc in range(KG):
                                nc.tensor.matmul(
                                    pg[:], wf_t[kc, m][:],
                                    rhs_gate[kc][:, js],
                                    start=(kc == 0), stop=(kc == KG - 1))

                        def emit_proj():
                            for kc in range(KU):
                                nc.tensor.matmul(
                                    pj[:], wp_t[kc, m][:], up[kc][:, js],
                                    start=(kc == 0), stop=(kc == KU - 1))

                        if proj_first:
                            emit_proj()
                            emit_gate()
                        else:
                            emit_gate()
                            emit_proj()
                        # g = sigmoid(gate_logits + b_forget)
                        g = mid.tile([128, BLK], f32, tag=f"g{m}")
                        nc.scalar.activation(
                            g[:], pg[:],
                            mybir.ActivationFunctionType.Sigmoid,
                            bias=bf_t[m][:])
                        # e = (proj + b_proj) - prev
                        e = mid.tile([128, BLK], f32, tag=f"e{m}")
                        pv_in = (pv[m][:, js] if PREV_FP16
                                 else pv[m][:, js].bitcast(f32))
                        nc.vector.scalar_tensor_tensor(
                            e[:], pj[:], bp_t[m][:], pv_in,
                            op0=ALU.add, op1=ALU.subtract)
                        # t2 = g * e
                        t2 = mid.tile([128, BLK], f32, tag=f"t{m}")
                        nc.vector.tensor_mul(t2[:], g[:], e[:])
                        # out = (proj+b_proj) - t2 = g*prev + (1-g)*(proj+b_p)
                        nc.vector.scalar_tensor_tensor(
                            ot[m][:, js], pj[:], bp_t[m][:], t2[:],
                            op0=ALU.add, op1=ALU.subtract)
                out_eng = nc.scalar if out_dma == "scalar" else nc.sync
                for m in range(NM):
                    out_eng.dma_start(
                        out_t[m * 128:(m + 1) * 128, s:s + w], ot[m][:, :w])

    nc.compile()
    return nc


def _get_nc(nblk=NBLK, repeat=1, **kw):
    key = (nblk, repeat, tuple(sorted(kw.items())))
    if key not in _CACHE:
        _CACHE[key] = _build_nc(nblk, repeat, **kw)
    return _CACHE[key]


def _prep_core_inputs(prev, upd, idx, wf, bf, wp, bp):
    """Build the 8 per-core input maps (host-side shard + transpose)."""
    in_maps = []
    p_np = np.float16 if PREV_FP16 else np.float32
    u_np = np.float16 if UPD_FP16 else np.float32
    for c in range(N_CORES):
        rows = idx[c * MC:(c + 1) * MC]
        xt_prev = np.zeros((D_NODE, MC_PAD), p_np)
        xt_prev[:, :MC] = prev[rows].T.astype(p_np)
        xt_upd = np.zeros((D_EXPR, MC_PAD), u_np)
        xt_upd[:, :MC] = upd[c * MC:(c + 1) * MC].T.astype(u_np)
        in_maps.append({
            "xt_prev": np.ascontiguousarray(xt_prev),
            "xt_upd": np.ascontiguousarray(xt_upd),
            "w_forget_p": np.ascontiguousarray(wf[:D_NODE].astype(p_np)),
            "w_forget_u": np.ascontiguousarray(wf[D_NODE:].astype(u_np)),
            "b_forget": bf,
            "w_proj": np.ascontiguousarray(wp.astype(u_np)),
            "b_proj": bp,
        })
    return in_maps


def _run_spmd(in_maps, trace=False):
    from concourse.bass_utils import run_bass_kernel_spmd
    nc = _get_nc()
    return run_bass_kernel_spmd(nc, in_maps, core_ids=list(range(N_CORES)),
                                trace=trace)


def measure_hw_time(rs=(3, 43), passes=4, runs=5, **build_kw):
    """Estimate the per-invocation device execution time of the kernel.

    neuron-profile NTFF capture isn't reachable through the axon tunnel in
    this container, so instead build the same program with the segment loop
    repeated R times inside one NEFF for two values of R and take the
    wall-clock slope between them — dispatch overhead cancels, leaving the
    device time per kernel iteration.  Cells are interleaved over several
    passes to cancel session drift."""
    import time
    import jax
    from jax.sharding import Mesh, PartitionSpec, NamedSharding
    from jax.experimental.shard_map import shard_map
    import concourse.mybir as mybir
    from concourse import bass2jax
    from concourse.bass2jax import _bass_exec_p, partition_id_tensor

    bass2jax.install_neuronx_cc_hook()
    devices = jax.devices()[:N_CORES]
    mesh = Mesh(np.asarray(devices), ("core",))
    spec = PartitionSpec("core")

    rng = np.random.default_rng(0)
    p_np = np.float16 if PREV_FP16 else np.float32
    u_np = np.float16 if UPD_FP16 else np.float32
    in_map = {
        "xt_prev": rng.standard_normal((D_NODE, MC_PAD)).astype(p_np),
        "xt_upd": rng.standard_normal((D_EXPR, MC_PAD)).astype(u_np),
        "w_forget_p": (rng.standard_normal((D_NODE, D_NODE))
                       .astype(p_np) / 32),
        "w_forget_u": (rng.standard_normal((D_EXPR, D_NODE))
                       .astype(u_np) / 32),
        "b_forget": np.zeros(D_NODE, np.float32),
        "w_proj": (rng.standard_normal((D_EXPR, D_NODE)).astype(u_np) / 32),
        "b_proj": np.zeros(D_NODE, np.float32),
    }

    def build_cell(R):
        nc = _build_nc(repeat=R, **build_kw)
        partition_name = (nc.partition_id_tensor.name
                          if nc.partition_id_tensor else None)
        in_names, out_names, out_avals, zero_outs = [], [], [], []
        for alloc in nc.m.functions[0].allocations:
            if not isinstance(alloc, mybir.MemoryLocationSet):
                continue
            name = alloc.memorylocations[0].name
            if alloc.kind == "ExternalInput":
                if name != partition_name:
                    in_names.append(name)
            elif alloc.kind == "ExternalOutput":
                out_names.append(name)
                shape = tuple(alloc.tensor_shape)
                dtype = mybir.dt.np(alloc.dtype)
                out_avals.append(jax.core.ShapedArray(shape, dtype))
                zero_outs.append(np.zeros(shape, dtype))
        n_params = len(in_names)
        n_outs = len(out_avals)
        all_in_names = list(in_names) + list(out_names)
        if partition_name is not None:
            all_in_names.append(partition_name)

        def _body(*args):
            operands = list(args)
            if partition_name is not None:
                operands.append(partition_id_tensor())
            return tuple(_bass_exec_p.bind(
                *operands, out_avals=tuple(out_avals),
                in_names=tuple(all_in_names), out_names=tuple(out_names),
                lowering_input_output_aliases=(),
                sim_require_finite=True, sim_require_nnan=True, nc=nc))

        sharded = jax.jit(
            shard_map(_body, mesh=mesh,
                      in_specs=(spec,) * (n_params + n_outs),
                      out_specs=(spec,) * n_outs, check_rep=False),
            keep_unused=True)
        staged = []
        for name in in_names:
            g = np.concatenate([in_map[name]] * N_CORES, axis=0)
            staged.append(jax.device_put(g, NamedSharding(mesh, spec)))
        for z in zero_outs:
            g = np.zeros((N_CORES * z.shape[0], *z.shape[1:]), z.dtype)
            staged.append(jax.device_put(g, NamedSharding(mesh, spec)))
        return sharded, staged

    def run_cell(cell, n):
        sharded, staged = cell
        ts = []
        for _ in range(n):
            t0 = time.perf_counter()
            o = sharded(*staged)
            jax.block_until_ready(o)
            _ = float(o[0].addressable_shards[0].data[0, 0])
            ts.append(time.perf_counter() - t0)
        return ts

    cells = {R: build_cell(R) for R in rs}
    for R in rs:
        run_cell(cells[R], 2)  # compile + warm
    samples = {R: [] for R in rs}
    for _ in range(passes):
        for R in rs:
            samples[R].extend(run_cell(cells[R], runs))
    meds = {}
    for R in rs:
        ts = sorted(samples[R])
        meds[R] = ts[len(ts) // 2]
    r1, r2 = min(rs), max(rs)
    return (meds[r2] - meds[r1]) / (r2 - r1)


def kernel(**inputs):
    prev = np.ascontiguousarray(
        np.asarray(inputs["previous_cfg_nodes_encodings"], np.float32))
    upd = np.ascontiguousarray(
        np.asarray(inputs["cfg_combined_expressions_encodings"], np.float32))
    mask = np.asarray(inputs["cfg_nodes_has_expression_mask"], bool)
    wf = np.ascontiguousarray(np.asarray(inputs["W_forget"], np.float32))
    bf = np.ascontiguousarray(np.asarray(inputs["b_forget"], np.float32))
    wp = np.ascontiguousarray(np.asarray(inputs["W_proj"], np.float32))
    bp = np.ascontiguousarray(np.asarray(inputs["b_proj"], np.float32))

    # mimic jnp.nonzero(mask, size=M, fill_value=0)
    idx = np.flatnonzero(mask)
    if idx.size >= M_ROWS:
        idx = idx[:M_ROWS]
    else:
        idx = np.concatenate(
            [idx, np.zeros(M_ROWS - idx.size, idx.dtype)])

    in_maps = _prep_core_inputs(prev, upd, idx, wf, bf, wp, bp)
    results = _run_spmd(in_maps).results

    out = prev.copy()
    for c in range(N_CORES):
        rows = idx[c * MC:(c + 1) * MC]
        out[rows] = results[c]["out_t"][:, :MC].T.astype(np.float32)
    return out



# revision 27
# speedup vs baseline: 32.5018x; 32.5018x over previous
"""Trainium2 Bass kernel for CFGNodeEncoderExpressionUpdateLayer.

Reference computation (per masked node row):
    idx  = nonzero(mask)                     # M rows of N
    prev = previous_cfg_nodes_encodings[idx]            # [M, 256]
    upd  = cfg_combined_expressions_encodings           # [M, 512]
    g    = sigmoid(concat(prev, upd) @ W_forget + b_forget)
    out_rows = g * prev + (1 - g) * (upd @ W_proj + b_proj)
    out  = previous_cfg_nodes_encodings; out[idx] = out_rows

Strategy:
  - Host: gather the M masked prev rows, shard rows across 8 cores
    (M/8 = 25000 rows each), transpose shards to [features, rows] so the
    device does only contiguous DMA and zero on-device transposes, pad
    rows to a multiple of 512.
  - Device (per core): load ~1MB column segments (2048 rows); for each
    512-row sub-block accumulate the two GEMMs over 128-deep K chunks in
    PSUM (prev in fp32r = fp32 bytes at 1 cycle/row; expressions and
    their weights in fp16 to halve their HBM traffic — the PE rounds
    fp32r to ~tf32 precision anyway), sigmoid+bias on ScalarE, and the
    gated blend out = pb - g*(pb - prev), pb = proj + b_proj, on VectorE
    via fused scalar_tensor_tensor ops.  prev stays fp32 on device; the
    blended output is written back as fp16 (host upcasts), cutting the
    output HBM traffic in half.  Keeping prev/gate operands f32r matters:
    fp16 prev measured consistently ~+50us slower (PE/DVE operand path),
    while fp16 out is a strict DMA win (77->64 MB per core).  Tile
    double-buffers everything; DMA-bound near the per-core HBM limit.
  - Host: transpose shards back and scatter into a copy of the full
    prev tensor.  Unmasked rows never touch the device.
"""

import numpy as np

N_NODES = 400000
M_ROWS = 200000
D_NODE = 256
D_EXPR = 512
D_CAT = D_NODE + D_EXPR

N_CORES = 8
MC = M_ROWS // N_CORES          # masked rows per core
BLK = 512                       # rows per block (matmul moving dim)
NBLK = -(-MC // BLK)            # 49
MC_PAD = NBLK * BLK             # 25088

_CACHE = {}
UPD_FP16 = True    # ship expressions + their weights as fp16
PREV_FP16 = False  # keep prev f32r: fp16 prev measured ~+50us (PE/DVE path)
OUT_FP16 = True    # return blended rows as fp16 (host upcasts); -13MB DMA


def _build_nc(nblk=NBLK, repeat=1, sup=1536, io_bufs=4, mid_bufs=3,
              psum_bufs=2, out_dma="scalar", upd_dma="sync",
              proj_first=True, hw_repeat=None,
              no_consumers=False, no_compute=False):
    import concourse.bacc as bacc
    import concourse.mybir as mybir
    import concourse.tile as tile

    f32 = mybir.dt.float32
    f32r = mybir.dt.float32r
    f16 = mybir.dt.float16
    u_dt = f16 if UPD_FP16 else f32r
    p_dt = f16 if PREV_FP16 else f32r
    o_dt = f16 if OUT_FP16 else f32
    ALU = mybir.AluOpType

    mc_pad = nblk * BLK
    nc = bacc.Bacc("TRN2", target_bir_lowering=False, debug=False,
                   num_devices=N_CORES)

    xt_prev = nc.dram_tensor("xt_prev", [D_NODE, mc_pad], p_dt,
                             kind="ExternalInput").ap()
    xt_upd = nc.dram_tensor("xt_upd", [D_EXPR, mc_pad], u_dt,
                            kind="ExternalInput").ap()
    w_forget_p = nc.dram_tensor("w_forget_p", [D_NODE, D_NODE], p_dt,
                                kind="ExternalInput").ap()
    w_forget_u = nc.dram_tensor("w_forget_u", [D_EXPR, D_NODE], u_dt,
                                kind="ExternalInput").ap()
    b_forget = nc.dram_tensor("b_forget", [D_NODE], f32,
                              kind="ExternalInput").ap()
    w_proj = nc.dram_tensor("w_proj", [D_EXPR, D_NODE], u_dt,
                            kind="ExternalInput").ap()
    b_proj = nc.dram_tensor("b_proj", [D_NODE], f32,
                            kind="ExternalInput").ap()
    out_t = nc.dram_tensor("out_t", [D_NODE, mc_pad], o_dt,
                           kind="ExternalOutput").ap()

    KP = D_NODE // 128   # 2 prev K-chunks
    KU = D_EXPR // 128   # 4 upd K-chunks
    KG = KP + KU         # 6 gate K-chunks
    NM = D_NODE // 128   # 2 output-feature chunks

    # column segments of the padded row space (big segments => big DMAs)
    mc_pad = nblk * BLK
    segs = []
    pos = 0
    while pos < mc_pad:
        w = min(sup, mc_pad - pos)
        segs.append((pos, w))
        pos += w

    with tile.TileContext(nc) as tc:
        with (
            tc.tile_pool(name="wpool", bufs=1) as wpool,
            tc.tile_pool(name="io", bufs=io_bufs) as io,
            tc.tile_pool(name="mid", bufs=mid_bufs) as mid,
            tc.tile_pool(name="psum", bufs=psum_bufs, space="PSUM") as pp,
        ):
            # --- preload weights & biases (stay resident in SBUF) ---
            wf_t = {}
            for kc in range(KG):
                for m in range(NM):
                    dt_w = p_dt if kc < KP else u_dt
                    src_w = (w_forget_p[kc * 128:(kc + 1) * 128,
                                        m * 128:(m + 1) * 128]
                             if kc < KP else
                             w_forget_u[(kc - KP) * 128:(kc - KP + 1) * 128,
                                        m * 128:(m + 1) * 128])
                    t = wpool.tile([128, 128], dt_w, tag=f"wf_{kc}_{m}")
                    nc.sync.dma_start(t[:], src_w)
                    wf_t[kc, m] = t
            wp_t = {}
            for kc in range(KU):
                for m in range(NM):
                    t = wpool.tile([128, 128], u_dt, tag=f"wp_{kc}_{m}")
                    nc.sync.dma_start(
                        t[:], w_proj[kc * 128:(kc + 1) * 128,
                                     m * 128:(m + 1) * 128])
                    wp_t[kc, m] = t
            bf_t, bp_t = {}, {}
            for m in range(NM):
                t = wpool.tile([128, 1], f32, tag=f"bf_{m}")
                nc.sync.dma_start(t[:], b_forget[m * 128:(m + 1) * 128]
                                  .unsqueeze(1))
                bf_t[m] = t
                t = wpool.tile([128, 1], f32, tag=f"bp_{m}")
                nc.sync.dma_start(t[:], b_proj[m * 128:(m + 1) * 128]
                                  .unsqueeze(1))
                bp_t[m] = t

            # --- main loop over column segments ---
            def emit_seg(s, w):
                pv = []
                for c in range(KP):
                    t = io.tile([128, sup], p_dt, tag=f"pv{c}")
                    nc.sync.dma_start(
                        t[:, :w], xt_prev[c * 128:(c + 1) * 128, s:s + w])
                    pv.append(t)
                up = []
                for c in range(KU):
                    if upd_dma == "scalar":
                        upd_eng = nc.scalar
                    elif upd_dma == "split":
                        upd_eng = nc.scalar if c % 2 else nc.sync
                    else:
                        upd_eng = nc.sync
                    t = io.tile([128, sup], u_dt, tag=f"up{c}")
                    upd_eng.dma_start(
                        t[:, :w], xt_upd[c * 128:(c + 1) * 128, s:s + w])
                    up.append(t)
                ot = [io.tile([128, sup], o_dt, tag=f"o{m}", name=f"ot{m}")
                      for m in range(NM)]
                rhs_gate = pv + up
                for j in range(w // BLK if not no_compute else 0):
                    js = slice(j * BLK, (j + 1) * BLK)
                    for m in range(NM):
                        pg = pp.tile([128, BLK], f32, tag=f"pg{m}")
                        pj = pp.tile([128, BLK], f32, tag=f"pp{m}")

                        def emit_gate():
                            for kc in range(KG):
                                nc.tensor.matmul(
                                    pg[:], wf_t[kc, m][:],
                                    rhs_gate[kc][:, js],
                                    start=(kc == 0), stop=(kc == KG - 1))

                        def emit_proj():
                            for kc in range(KU):
                                nc.tensor.matmul(
                                    pj[:], wp_t[kc, m][:], up[kc][:, js],
                                    start=(kc == 0), stop=(kc == KU - 1))

                        if proj_first:
                            emit_proj()
                            emit_gate()
                        else:
                            emit_gate()
                            emit_proj()
                        if no_consumers:
                            continue
                        # g = sigmoid(gate_logits + b_forget)
                        g = mid.tile([128, BLK], f32, tag=f"g{m}")
                        nc.scalar.activation(
                            g[:], pg[:],
                            mybir.ActivationFunctionType.Sigmoid,
                            bias=bf_t[m][:])
                        # e = (proj + b_proj) - prev
                        e = mid.tile([128, BLK], f32, tag=f"e{m}")
                        pv_in = (pv[m][:, js] if PREV_FP16
                                 else pv[m][:, js].bitcast(f32))
                        nc.vector.scalar_tensor_tensor(
                            e[:], pj[:], bp_t[m][:], pv_in,
                            op0=ALU.add, op1=ALU.subtract)
                        # t2 = g * e
                        t2 = mid.tile([128, BLK], f32, tag=f"t{m}")
                        nc.vector.tensor_mul(t2[:], g[:], e[:])
                        # out = (proj+b_proj) - t2 = g*prev + (1-g)*(proj+b_p)
                        nc.vector.scalar_tensor_tensor(
                            ot[m][:, js], pj[:], bp_t[m][:], t2[:],
                            op0=ALU.add, op1=ALU.subtract)
                out_eng = nc.scalar if out_dma == "scalar" else nc.sync
                if no_compute:
                    # dma-only diagnostic: same out bytes, sourced from pv
                    for m in range(NM):
                        out_eng.dma_start(
                            out_t[m * 128:(m + 1) * 128, s:s + w],
                            pv[m][:, :w].bitcast(o_dt))
                elif not no_consumers:
                    for m in range(NM):
                        out_eng.dma_start(
                            out_t[m * 128:(m + 1) * 128, s:s + w],
                            ot[m][:, :w])

            if hw_repeat:
                with tc.For_i(0, hw_repeat):
                    for s, w in segs:
                        emit_seg(s, w)
            else:
                for it in range(len(segs) * repeat):
                    emit_seg(*segs[it % len(segs)])

    nc.compile()
    return nc


def _get_nc(nblk=NBLK, repeat=1, **kw):
    key = (nblk, repeat, tuple(sorted(kw.items())))
    if key not in _CACHE:
        _CACHE[key] = _build_nc(nblk, repeat, **kw)
    return _CACHE[key]


def _prep_core_inputs(prev, upd, idx, wf, bf, wp, bp):
    """Build the 8 per-core input maps (host-side shard + transpose)."""
    in_maps = []
    p_np = np.float16 if PREV_FP16 else np.float32
    u_np = np.float16 if UPD_FP16 else np.float32
    for c in range(N_CORES):
        rows = idx[c * MC:(c + 1) * MC]
        xt_prev = np.zeros((D_NODE, MC_PAD), p_np)
        xt_prev[:, :MC] = prev[rows].T.astype(p_np)
        xt_upd = np.zeros((D_EXPR, MC_PAD), u_np)
        xt_upd[:, :MC] = upd[c * MC:(c + 1) * MC].T.astype(u_np)
        in_maps.append({
            "xt_prev": np.ascontiguousarray(xt_prev),
            "xt_upd": np.ascontiguousarray(xt_upd),
            "w_forget_p": np.ascontiguousarray(wf[:D_NODE].astype(p_np)),
            "w_forget_u": np.ascontiguousarray(wf[D_NODE:].astype(u_np)),
            "b_forget": bf,
            "w_proj": np.ascontiguousarray(wp.astype(u_np)),
            "b_proj": bp,
        })
    return in_maps


def _run_spmd(in_maps, trace=False):
    from concourse.bass_utils import run_bass_kernel_spmd
    nc = _get_nc()
    return run_bass_kernel_spmd(nc, in_maps, core_ids=list(range(N_CORES)),
                                trace=trace)


def measure_hw_time(rs=(3, 43), passes=4, runs=5, protocol="unroll",
                    **build_kw):
    """Estimate the per-invocation device execution time of the kernel.

    neuron-profile NTFF capture isn't reachable through the axon tunnel in
    this container, so instead build the same program with the segment loop
    wrapped in a For_i hardware loop of R iterations for two values of R
    and take the wall-clock slope between them — dispatch overhead cancels,
    leaving the device time per kernel iteration.  The hardware loop keeps
    the program small, so R can be large enough (~200) that the ~40ms
    wall-clock contrast dwarfs the ~1ms dispatch jitter.  Cells are
    interleaved over several passes to cancel session drift."""
    import time
    import jax
    from jax.sharding import Mesh, PartitionSpec, NamedSharding
    from jax.experimental.shard_map import shard_map
    import concourse.mybir as mybir
    from concourse import bass2jax
    from concourse.bass2jax import _bass_exec_p, partition_id_tensor

    bass2jax.install_neuronx_cc_hook()
    devices = jax.devices()[:N_CORES]
    mesh = Mesh(np.asarray(devices), ("core",))
    spec = PartitionSpec("core")

    rng = np.random.default_rng(0)
    p_np = np.float16 if PREV_FP16 else np.float32
    u_np = np.float16 if UPD_FP16 else np.float32
    in_map = {
        "xt_prev": rng.standard_normal((D_NODE, MC_PAD)).astype(p_np),
        "xt_upd": rng.standard_normal((D_EXPR, MC_PAD)).astype(u_np),
        "w_forget_p": (rng.standard_normal((D_NODE, D_NODE))
                       .astype(p_np) / 32),
        "w_forget_u": (rng.standard_normal((D_EXPR, D_NODE))
                       .astype(u_np) / 32),
        "b_forget": np.zeros(D_NODE, np.float32),
        "w_proj": (rng.standard_normal((D_EXPR, D_NODE)).astype(u_np) / 32),
        "b_proj": np.zeros(D_NODE, np.float32),
    }

    def build_cell(R):
        if protocol == "fori":
            nc = _build_nc(hw_repeat=R, **build_kw)
        else:
            nc = _build_nc(repeat=R, **build_kw)
        partition_name = (nc.partition_id_tensor.name
                          if nc.partition_id_tensor else None)
        in_names, out_names, out_avals, zero_outs = [], [], [], []
        for alloc in nc.m.functions[0].allocations:
            if not isinstance(alloc, mybir.MemoryLocationSet):
                continue
            name = alloc.memorylocations[0].name
            if alloc.kind == "ExternalInput":
                if name != partition_name:
                    in_names.append(name)
            elif alloc.kind == "ExternalOutput":
                out_names.append(name)
                shape = tuple(alloc.tensor_shape)
                dtype = mybir.dt.np(alloc.dtype)
                out_avals.append(jax.core.ShapedArray(shape, dtype))
                zero_outs.append(np.zeros(shape, dtype))
        n_params = len(in_names)
        n_outs = len(out_avals)
        all_in_names = list(in_names) + list(out_names)
        if partition_name is not None:
            all_in_names.append(partition_name)

        def _body(*args):
            operands = list(args)
            if partition_name is not None:
                operands.append(partition_id_tensor())
            return tuple(_bass_exec_p.bind(
                *operands, out_avals=tuple(out_avals),
                in_names=tuple(all_in_names), out_names=tuple(out_names),
                lowering_input_output_aliases=(),
                sim_require_finite=True, sim_require_nnan=True, nc=nc))

        sharded = jax.jit(
            shard_map(_body, mesh=mesh,
                      in_specs=(spec,) * (n_params + n_outs),
                      out_specs=(spec,) * n_outs, check_rep=False),
            keep_unused=True)
        staged = []
        for name in in_names:
            g = np.concatenate([in_map[name]] * N_CORES, axis=0)
            staged.append(jax.device_put(g, NamedSharding(mesh, spec)))
        for z in zero_outs:
            g = np.zeros((N_CORES * z.shape[0], *z.shape[1:]), z.dtype)
            staged.append(jax.device_put(g, NamedSharding(mesh, spec)))
        return sharded, staged

    def run_cell(cell, n):
        sharded, staged = cell
        ts = []
        for _ in range(n):
            t0 = time.perf_counter()
            o = sharded(*staged)
            jax.block_until_ready(o)
            _ = float(o[0].addressable_shards[0].data[0, 0])
            ts.append(time.perf_counter() - t0)
        return ts

    cells = {R: build_cell(R) for R in rs}
    for R in rs:
        run_cell(cells[R], 2)  # compile + warm
    samples = {R: [] for R in rs}
    for _ in range(passes):
        for R in rs:
            samples[R].extend(run_cell(cells[R], runs))
    meds = {}
    for R in rs:
        ts = sorted(samples[R])
        meds[R] = ts[len(ts) // 2]
    r1, r2 = min(rs), max(rs)
    return (meds[r2] - meds[r1]) / (r2 - r1)


def kernel(**inputs):
    prev = np.ascontiguousarray(
        np.asarray(inputs["previous_cfg_nodes_encodings"], np.float32))
    upd = np.ascontiguousarray(
        np.asarray(inputs["cfg_combined_expressions_encodings"], np.float32))
    mask = np.asarray(inputs["cfg_nodes_has_expression_mask"], bool)
    wf = np.ascontiguousarray(np.asarray(inputs["W_forget"], np.float32))
    bf = np.ascontiguousarray(np.asarray(inputs["b_forget"], np.float32))
    wp = np.ascontiguousarray(np.asarray(inputs["W_proj"], np.float32))
    bp = np.ascontiguousarray(np.asarray(inputs["b_proj"], np.float32))

    # mimic jnp.nonzero(mask, size=M, fill_value=0)
    idx = np.flatnonzero(mask)
    if idx.size >= M_ROWS:
        idx = idx[:M_ROWS]
    else:
        idx = np.concatenate(
            [idx, np.zeros(M_ROWS - idx.size, idx.dtype)])

    in_maps = _prep_core_inputs(prev, upd, idx, wf, bf, wp, bp)
    results = _run_spmd(in_maps).results

    out = prev.copy()
    for c in range(N_CORES):
        rows = idx[c * MC:(c + 1) * MC]
        out[rows] = results[c]["out_t"][:, :MC].T.astype(np.float32)
    return out

